# revision 23
# baseline (speedup 1.0000x reference)
"""Multi-head causal self-attention (B=2, N=2048, D=2048, H=16) on 8 NeuronCores.

Sharding: core c handles batch b = c//4 and heads 4*(c%4) .. 4*(c%4)+3
(data parallel over batch, tensor parallel over heads).  Each core:
  - computes the qkv projection for its 4-head column slice of W_qkv,
    keeping Q^T / K^T in [head_dim, seq] layout and V in natural [seq, head_dim]
    (x is pre-transposed and partition-major-blocked on the host so each
    stage-1 tensor loads with one fully contiguous dma_start),
  - runs causal attention per head entirely in transposed space
    (S^T = K_tile Q^T, exp on ScalarE writes P^T straight into SBUF,
    denominators via ones-row matmuls, 1/sum fused into the ctx copy),
  - computes the partial output projection ctx_slice @ W_out[rows_slice]
    into a [2048, 2048] bf16 partial.
The host sums the 4 partials per batch (fp64) and adds the output bias.

Performance structure (vs the 390us baseline; measured ~348us):
  - x lives in HBM as [quarter][128p][chunk*512] so stage-1 DMAs are one
    contiguous dma_start per tensor/quarter; wq and the first x quarter are
    interleaved in 4-chunk sub-transfers so the PE's first psum group
    progresses as data lands (no >3.4us idle, HAM clock gate stays at 8/8).
  - x is staged through a 2-deep quarter ring; V seq tiles run right after
    their QK quarter so each quarter dies early.
  - scores for groups 0/1 (heads 0-1) are emitted inside stage 1, so their
    exp work on ScalarE is done long before the attention drains need it --
    the stage-1 -> stage-2 transition has no exp warm-up bubble.
  - the partition_broadcast Q7 library and the exp table are primed at t=0
    (the lazy library load otherwise costs ~10.5us at the stage handoff);
    exp processes two full 512 blocks per ACTIVATE via [128, 2, 512]
    two-bank psum tiles; out-proj PSUM evacuation alternates VectorE/ScalarE
    and the output partial is written bf16 in a blocked HBM layout.

Matmul inputs are bf16 (fp32 accumulation in PSUM); measured end-to-end
relative error vs the fp32 reference is ~6e-3.
"""

import math

import numpy as np
import ml_dtypes

import concourse.bass as bass
import concourse.mybir as mybir
import concourse.tile as tile
from concourse import bacc
from concourse.bass_utils import run_bass_kernel_spmd

BF16 = mybir.dt.bfloat16
F32 = mybir.dt.float32
AX = mybir.AxisListType
ALU = mybir.AluOpType
ACT_EXP = mybir.ActivationFunctionType.Exp

P = 128              # partitions
D_IN = 2048          # model dim
N_SEQ = 2048         # sequence length
HD = 128             # head dim
HPC = 4              # heads per core
DC = HPC * HD        # 512: d_out slice per core
N_CORES = 8
NQ = 4               # 512-col quarters of the sequence
SCALE = 1.0 / math.sqrt(HD)
NEG_BIG = -1e10


def _build_body(tc, xt_d, wq_d, wk_d, wv_d, wo_d, out_d, n_seq=N_SEQ):
    nc = tc.nc
    NT = n_seq // P        # 16 seq tiles of 128
    NI = D_IN // P         # 16 contraction chunks of 128
    NJ = D_IN // 512       # 4 output column chunks

    from contextlib import ExitStack
    ctx = ExitStack()
    with ctx:
        const = ctx.enter_context(tc.tile_pool(name="const", bufs=1))
        # transposed causal mask for S^T blocks: keep k <= q
        mask2 = const.tile([P, P], F32)
        nc.gpsimd.memset(mask2, 0.0)
        nc.gpsimd.affine_select(
            out=mask2, in_=mask2, compare_op=ALU.is_ge, fill=NEG_BIG,
            base=0, pattern=[[1, P]], channel_multiplier=-1,
        )
        ones_sb = const.tile([P, 1], BF16)
        nc.vector.memset(ones_sb, 1.0)
        warmsrc = const.tile([P, 512], BF16)
        nc.vector.memset(warmsrc, 0.0)
        # force the exp activation table load at t=0, off the critical path
        tscr = const.tile([P, 1], F32)
        nc.vector.memset(tscr, 0.0)
        nc.scalar.activation(out=tscr, in_=tscr, func=ACT_EXP, bias=0.0, scale=1.0)
        # force the partition_broadcast Q7 library load at t=0 as well --
        # lazily it costs ~10.5us right at the stage-1 -> stage-2 handoff.
        brow = const.tile([1, 128], F32)
        nc.vector.memset(brow, 0.0)
        bwarm = const.tile([P, 128], F32)
        nc.gpsimd.partition_broadcast(bwarm, brow)

        # activations that persist across stages
        persist = ctx.enter_context(tc.tile_pool(name="persist", bufs=1))
        qt_sb = persist.tile([P, HPC, n_seq], BF16)    # Q^T  [d, h, n]
        kt_sb = persist.tile([P, HPC, n_seq], BF16)    # K^T  [d, h, n]
        v_sb = persist.tile([P, NT, DC], BF16)         # V natural [n(128), nt, d]
        ctxT_sb = persist.tile([P, HPC, n_seq], BF16)  # ctx^T [d, h, n]

        # scores psum pool spans both stages (4 banks)
        s_pool = ctx.enter_context(tc.tile_pool(name="sps", bufs=2, space="PSUM"))
        # ptg pool for groups 0/1 spans both stages (6 x 8KB)
        att1 = ctx.enter_context(tc.tile_pool(name="att1", bufs=6))

        ptgs = {}

        def emit_scores(g, h, pool):
            """S^T = K^T_kt.T @ Q^T per k tile, mask diagonal, exp into the
            P^T group tile.  Off-diagonal blocks run in pairs through a
            [128, 2, 512] two-bank psum tile and one ACTIVATE; the 4
            diagonal blocks share pair tiles half/half."""
            nkt = 4 * (g + 1)
            q0 = 4 * g * P
            ptg = pool.tile([P, nkt, 512], BF16, tag="ptg",
                            name=f"ptg_{g}_{h}")
            for kt0 in range(0, nkt, 2):
                sps = s_pool.tile([P, 2, 512], F32, tag="s", name="sps")
                if kt0 + 1 < 4 * g:
                    for j in (0, 1):
                        nc.tensor.matmul(
                            sps[:, j, :],
                            lhsT=kt_sb[:, h, (kt0 + j) * P:(kt0 + j + 1) * P],
                            rhs=qt_sb[:, h, q0:q0 + 512],
                            start=True, stop=True,
                        )
                    nc.scalar.activation(
                        out=ptg[:, kt0:kt0 + 2, :], in_=sps,
                        func=ACT_EXP, bias=0.0, scale=SCALE,
                    )
                else:
                    for j in (0, 1):
                        kt = kt0 + j
                        off = max(kt - 4 * g, 0) * P
                        width = 512 - off
                        nc.tensor.matmul(
                            sps[:, j, :width],
                            lhsT=kt_sb[:, h, kt * P:(kt + 1) * P],
                            rhs=qt_sb[:, h, q0 + off:q0 + 512],
                            start=True, stop=True,
                        )
                        if kt >= 4 * g:
                            nc.vector.tensor_tensor(
                                out=sps[:, j, :P], in0=sps[:, j, :P],
                                in1=mask2, op=ALU.add,
                            )
                        nc.scalar.activation(
                            out=ptg[:, kt, off:512], in_=sps[:, j, :width],
                            func=ACT_EXP, bias=0.0, scale=SCALE,
                        )
            return ptg

        def emit_sp(g, hp, pool):
            ptgs[(g, hp)] = emit_scores(g, hp, pool)
            ptgs[(g, hp + 1)] = emit_scores(g, hp + 1, pool)

        # ---------------- stage 1: qkv projection ----------------
        with tc.tile_pool(name="xw", bufs=1) as xw_pool, \
             tc.tile_pool(name="qkps", bufs=2, space="PSUM") as qk_pool, \
             tc.tile_pool(name="vps", bufs=2, space="PSUM") as v_pool:
            wq_sb = xw_pool.tile([P, NI, DC], BF16)
            wk_sb = xw_pool.tile([P, NI, DC], BF16)
            wv_sb = xw_pool.tile([P, NI, DC], BF16)
            # Dummy matmuls with no input dependencies: ramp the HAM clock
            # gate while the first DMAs land.
            warm_ps = v_pool.tile([P, 512], F32, tag="v", name="warm_ps")
            for _ in range(24):
                nc.tensor.matmul(warm_ps, lhsT=warmsrc[:, :P], rhs=warmsrc,
                                 start=True, stop=True)

            # Everything is partition-major in HBM, so each transfer below is
            # one dma_start whose per-partition runs are fully contiguous.
            xt_r = xt_d.rearrange("(q p) (io c) -> q p io c", q=NQ, io=NI)
            wq_r = wq_d.rearrange("p (io c) -> p io c", io=NI)
            wk_r = wk_d.rearrange("p (io c) -> p io c", io=NI)
            wv_r = wv_d.rearrange("p (io c) -> p io c", io=NI)

            xtq = [None] * NQ

            def xq_dma(q):
                t = xw_pool.tile([P, NI, 512], BF16, tag="xtq", bufs=2,
                                 name=f"xtq{q}")
                nc.sync.dma_start(t, xt_r[q])
                xtq[q] = t

            # DMA in consumption order (HWDGE ring drains FIFO).  wq and the
            # first x quarter are interleaved in 4-chunk sub-transfers so the
            # first QK psum group progresses chunk-by-chunk as data lands.
            t0 = xw_pool.tile([P, NI, 512], BF16, tag="xtq", bufs=2,
                              name="xtq0")
            xtq[0] = t0
            for i4 in range(0, NI, 4):
                s4 = slice(i4, i4 + 4)
                nc.sync.dma_start(wq_sb[:, s4, :], wq_r[:, s4, :])
                nc.sync.dma_start(t0[:, s4, :], xt_r[0, :, s4, :])
            nc.sync.dma_start(wk_sb, wk_r)
            nc.sync.dma_start(wv_sb, wv_r)
            for q in range(1, NQ):
                xq_dma(q)

            for q4 in range(NQ):
                xq = xtq[q4]
                sl = slice(q4 * 512, (q4 + 1) * 512)
                # Q^T / K^T for this quarter
                for w_sb, dst in ((wq_sb, qt_sb), (wk_sb, kt_sb)):
                    for h in range(HPC):
                        ps = qk_pool.tile([P, 512], F32, tag="qk")
                        for i in range(NI):
                            nc.tensor.matmul(
                                ps,
                                lhsT=w_sb[:, i, h * P:(h + 1) * P],
                                rhs=xq[:, i, :],
                                start=(i == 0), stop=(i == NI - 1),
                            )
                        nc.vector.tensor_copy(out=dst[:, h, sl], in_=ps)
                # V seq tiles of this quarter
                for j in range(4):
                    nt = 4 * q4 + j
                    ps = v_pool.tile([P, DC], F32, tag="v")
                    for i in range(NI):
                        nc.tensor.matmul(
                            ps,
                            lhsT=xq[:, i, j * P:(j + 1) * P],
                            rhs=wv_sb[:, i, :],
                            start=(i == 0), stop=(i == NI - 1),
                        )
                    nc.vector.tensor_copy(out=v_sb[:, nt, :], in_=ps)
                # early scores for groups 0/1: their exp work runs on
                # ScalarE underneath the remaining QK/V matmuls.
                if q4 == 0:
                    emit_sp(0, 0, att1)
                elif q4 == 1:
                    emit_sp(0, 2, att1)
                elif q4 == 2:
                    emit_sp(1, 0, att1)

        # ---------------- stage 2: attention drains + out-projection ------
        with tc.tile_pool(name="att2", bufs=4) as att2, \
             tc.tile_pool(name="att_small", bufs=2) as small_pool, \
             tc.tile_pool(name="colps", bufs=1, space="PSUM") as col_pool, \
             tc.tile_pool(name="cps", bufs=1, space="PSUM") as c_pool, \
             tc.tile_pool(name="out_sb", bufs=3) as out_pool, \
             tc.tile_pool(name="wo_sb", bufs=1) as wo_pool, \
             tc.tile_pool(name="ops", bufs=2, space="PSUM") as o_pool:
            wo_sb = wo_pool.tile([P, HPC, D_IN], BF16)
            wo_r = wo_d.rearrange("p (h j) -> p h j", h=HPC)
            nc.sync.dma_start(wo_sb, wo_r)
            out_r = out_d.rearrange("(nt jc p) c -> nt jc p c", nt=NT, jc=NJ)

            def emit_dr(g, hp):
                """Drain a head pair: sequential ones-row colsum chains in
                one PSUM bank, reciprocal, partition-broadcast, then ctx
                accumulation with the 1/sum fused into the PSUM->SBUF copy."""
                nkt = 4 * (g + 1)
                colp = col_pool.tile([P, 512], F32, tag="col", name="colp")
                btile = o_pool.tile([P, 512], F32, tag="o", name="colp_b")
                pa, pb = ptgs.pop((g, hp)), ptgs.pop((g, hp + 1))
                # col-tiled chains in separate banks: A -> col group 0 /
                # partition 0, B -> col group 1 / partition 32; interleaved
                # kt-major so the two chains stream concurrently.
                # (reciprocal_approx_fast garbles partition-offset inputs, so
                # chain B's row is tensor_copy'd to partition 0 first.)
                for kt in range(nkt):
                    off = max(kt - 4 * g, 0) * P
                    nc.tensor.matmul(
                        colp[0:1, off:512],
                        lhsT=ones_sb, rhs=pa[:, kt, off:512],
                        start=(kt == 0), stop=(kt == nkt - 1),
                        skip_group_check=True, tile_position=(0, 0),
                    )
                    nc.tensor.matmul(
                        btile[32:33, off:512],
                        lhsT=ones_sb, rhs=pb[:, kt, off:512],
                        start=(kt == 0), stop=(kt == nkt - 1),
                        skip_group_check=True, tile_position=(0, 32),
                    )
                bcs = []
                for cg, dst in ((0, colp), (1, btile)):
                    recip = small_pool.tile([1, 512], F32, tag="rsb",
                                            name="recip_sb")
                    if cg == 0:
                        nc.vector.reciprocal_approx_fast(out=recip,
                                                         in_=dst[0:1, :])
                    else:
                        nc.vector.tensor_copy(out=recip, in_=dst[32:33, :])
                        nc.vector.reciprocal_approx_fast(out=recip, in_=recip)
                    bc = small_pool.tile([P, 512], F32, tag="rbc",
                                         name="recip_bc")
                    nc.gpsimd.partition_broadcast(bc, recip)
                    bcs.append(bc)
                for h, pt, bc in ((hp, pa, bcs[0]), (hp + 1, pb, bcs[1])):
                    cps = c_pool.tile([P, 512], F32, tag="c", name="cps")
                    for kt in range(nkt):
                        off = max(kt - 4 * g, 0) * P
                        nc.tensor.matmul(
                            cps[:, off:512],
                            lhsT=v_sb[:, kt, h * P:(h + 1) * P],
                            rhs=pt[:, kt, off:512],
                            start=(kt == 0), stop=(kt == nkt - 1),
                            skip_group_check=True,
                        )
                    nc.vector.tensor_tensor(
                        out=ctxT_sb[:, h, g * 512:(g + 1) * 512],
                        in0=cps, in1=bc, op=ALU.mult,
                    )

            def emit_op(nts):
                """Partial out-projection for seq tiles `nts`; PSUM
                evacuation alternates VectorE / ScalarE; bf16 out."""
                for idx, nt in enumerate(nts):
                    for jc in range(NJ):
                        ops = o_pool.tile([P, 512], F32, tag="o", name="ops")
                        for hh in range(HPC):
                            nc.tensor.matmul(
                                ops,
                                lhsT=ctxT_sb[:, hh, nt * P:(nt + 1) * P],
                                rhs=wo_sb[:, hh, jc * 512:(jc + 1) * 512],
                                start=(hh == 0), stop=(hh == HPC - 1),
                            )
                        osb = out_pool.tile([P, 512], BF16, tag="osb",
                                            name="osb")
                        if (idx * NJ + jc) % 2:
                            nc.scalar.copy(out=osb, in_=ops)
                        else:
                            nc.vector.tensor_copy(out=osb, in_=ops)
                        nc.sync.dma_start(out_r[nt, jc], osb)

            # Explicit stage-2 schedule: drains for groups 0/1 run
            # immediately (their exps are already done); score pairs for
            # groups 2/3 and out-proj chunks are interleaved so every
            # drain's exp work is covered by preceding PE work.
            emit_dr(0, 0)
            emit_dr(0, 2)
            emit_sp(1, 2, att1)
            emit_op(range(0, 4))
            emit_sp(2, 0, att2)
            emit_dr(1, 0)
            emit_sp(2, 2, att2)
            emit_dr(1, 2)
            emit_op(range(4, 6))
            emit_sp(3, 0, att2)
            emit_dr(2, 0)
            emit_sp(3, 2, att2)
            emit_dr(2, 2)
            emit_op(range(8, 12))
            emit_dr(3, 0)
            emit_op(range(6, 8))
            emit_dr(3, 2)
            emit_op(range(12, 16))


def build_module(n_seq=N_SEQ):
    """Build and compile the per-core Bass module (SPMD: same program, 8 cores)."""
    nc = bacc.Bacc("TRN2", target_bir_lowering=False, debug=False,
                   num_devices=N_CORES)
    NI = D_IN // P
    xt_d = nc.dram_tensor("xt", [NQ * P, NI * 512], BF16,
                          kind="ExternalInput").ap()
    wq_d = nc.dram_tensor("wq", [P, NI * DC], BF16, kind="ExternalInput").ap()
    wk_d = nc.dram_tensor("wk", [P, NI * DC], BF16, kind="ExternalInput").ap()
    wv_d = nc.dram_tensor("wv", [P, NI * DC], BF16, kind="ExternalInput").ap()
    wo_d = nc.dram_tensor("wo", [P, HPC * D_IN], BF16, kind="ExternalInput").ap()
    out_d = nc.dram_tensor("out", [n_seq * (D_IN // 512), 512], BF16,
                           kind="ExternalOutput").ap()
    with tile.TileContext(nc) as tc:
        _build_body(tc, xt_d, wq_d, wk_d, wv_d, wo_d, out_d, n_seq)
    nc.compile()
    return nc


def make_in_maps(x, W_qkv, W_out):
    """Host-side sharding: per-core input dict, bf16 cast.  x is blocked as
    [quarter q][chunk io][128 p][512 c] = x[b].T[io*128+p, q*512+c] so every
    stage-1 DMA descriptor is one contiguous 128KB block."""
    bf = ml_dtypes.bfloat16
    NI = D_IN // P

    def wblk(w):
        # [d_in, c] -> [p, io*c]: partition-major, contiguous per partition
        return np.ascontiguousarray(
            w.reshape(NI, P, -1).transpose(1, 0, 2).reshape(P, -1)).astype(bf)

    in_maps = []
    for c in range(N_CORES):
        b, g = divmod(c, 4)
        cs = slice(DC * g, DC * (g + 1))
        xT = np.asarray(x[b]).T.astype(bf)                    # [d, n]
        # [q*p, io*c]: quarter-then-partition major
        xq = xT.reshape(NI, P, NQ, 512).transpose(2, 1, 0, 3).reshape(NQ * P, -1)
        wo = np.asarray(W_out[cs, :]).reshape(HPC, P, D_IN).transpose(1, 0, 2)
        in_maps.append({
            "xt": np.ascontiguousarray(xq),
            "wq": wblk(W_qkv[:, 0 * D_IN:1 * D_IN][:, cs]),
            "wk": wblk(W_qkv[:, 1 * D_IN:2 * D_IN][:, cs]),
            "wv": wblk(W_qkv[:, 2 * D_IN:3 * D_IN][:, cs]),
            "wo": np.ascontiguousarray(wo.reshape(P, -1)).astype(bf),
        })
    return in_maps


_NC_CACHE = {}


def get_module():
    if "nc" not in _NC_CACHE:
        _NC_CACHE["nc"] = build_module()
    return _NC_CACHE["nc"]


def run(x, W_qkv, W_out, b_out, trace=False, **trace_kwargs):
    nc = get_module()
    in_maps = make_in_maps(x, W_qkv, W_out)
    res = run_bass_kernel_spmd(nc, in_maps, core_ids=list(range(N_CORES)),
                               trace=trace, **trace_kwargs)
    NT, NJ = N_SEQ // P, D_IN // 512
    parts = np.stack([
        np.asarray(res.results[c]["out"], dtype=np.float32)
        .reshape(NT, NJ, P, 512).transpose(0, 2, 1, 3).reshape(N_SEQ, D_IN)
        for c in range(N_CORES)])
    parts = parts.reshape(2, 4, N_SEQ, D_IN)
    out = parts.sum(axis=1, dtype=np.float64).astype(np.float32)
    out += b_out.astype(np.float32)
    return out, res


def kernel(x, W_qkv, W_out, b_out):
    out, _ = run(np.asarray(x), np.asarray(W_qkv), np.asarray(W_out),
                 np.asarray(b_out))
    return out


# revision 24
# speedup vs baseline: 1.0063x; 1.0063x over previous
"""Multi-head causal self-attention (B=2, N=2048, D=2048, H=16) on 8 NeuronCores.

Sharding: core c handles batch b = c//4 and heads 4*(c%4) .. 4*(c%4)+3
(data parallel over batch, tensor parallel over heads).  Each core:
  - computes the qkv projection for its 4-head column slice of W_qkv,
    keeping Q^T / K^T in [head_dim, seq] layout and V in natural [seq, head_dim]
    (x is pre-transposed and partition-major-blocked on the host so each
    stage-1 tensor loads with one fully contiguous dma_start),
  - runs causal attention per head entirely in transposed space
    (S^T = K_tile Q^T, exp on ScalarE writes P^T straight into SBUF,
    denominators via ones-row matmuls, 1/sum fused into the ctx copy),
  - computes the partial output projection ctx_slice @ W_out[rows_slice]
    into a [2048, 2048] bf16 partial.
The host sums the 4 partials per batch (fp64) and adds the output bias.

Performance structure (vs the 390us baseline; measured ~348us):
  - x lives in HBM as [quarter][128p][chunk*512] so stage-1 DMAs are one
    contiguous dma_start per tensor/quarter; wq and the first x quarter are
    interleaved in 4-chunk sub-transfers so the PE's first psum group
    progresses as data lands (no >3.4us idle, HAM clock gate stays at 8/8).
  - x is staged through a 2-deep quarter ring; V seq tiles run right after
    their QK quarter so each quarter dies early.
  - scores for groups 0/1 (heads 0-1) are emitted inside stage 1, so their
    exp work on ScalarE is done long before the attention drains need it --
    the stage-1 -> stage-2 transition has no exp warm-up bubble.
  - the partition_broadcast Q7 library and the exp table are primed at t=0
    (the lazy library load otherwise costs ~10.5us at the stage handoff);
    exp processes two full 512 blocks per ACTIVATE via [128, 2, 512]
    two-bank psum tiles; out-proj PSUM evacuation alternates VectorE/ScalarE
    and the output partial is written bf16 in a blocked HBM layout.

Matmul inputs are bf16 (fp32 accumulation in PSUM); measured end-to-end
relative error vs the fp32 reference is ~6e-3.
"""

import math

import numpy as np
import ml_dtypes

import concourse.bass as bass
import concourse.mybir as mybir
import concourse.tile as tile
from concourse import bacc
from concourse.bass_utils import run_bass_kernel_spmd

BF16 = mybir.dt.bfloat16
F32 = mybir.dt.float32
AX = mybir.AxisListType
ALU = mybir.AluOpType
ACT_EXP = mybir.ActivationFunctionType.Exp

P = 128              # partitions
D_IN = 2048          # model dim
N_SEQ = 2048         # sequence length
HD = 128             # head dim
HPC = 4              # heads per core
DC = HPC * HD        # 512: d_out slice per core
N_CORES = 8
NQ = 4               # 512-col quarters of the sequence
SCALE = 1.0 / math.sqrt(HD)
NEG_BIG = -1e10


def _build_body(tc, xt_d, wq_d, wk_d, wv_d, wo_d, out_d, n_seq=N_SEQ):
    nc = tc.nc
    NT = n_seq // P        # 16 seq tiles of 128
    NI = D_IN // P         # 16 contraction chunks of 128
    NJ = D_IN // 512       # 4 output column chunks

    from contextlib import ExitStack
    ctx = ExitStack()
    with ctx:
        const = ctx.enter_context(tc.tile_pool(name="const", bufs=1))
        # transposed causal mask for S^T blocks: keep k <= q
        mask2 = const.tile([P, P], F32)
        nc.gpsimd.memset(mask2, 0.0)
        nc.gpsimd.affine_select(
            out=mask2, in_=mask2, compare_op=ALU.is_ge, fill=NEG_BIG,
            base=0, pattern=[[1, P]], channel_multiplier=-1,
        )
        ones_sb = const.tile([P, 1], BF16)
        nc.vector.memset(ones_sb, 1.0)
        warmsrc = const.tile([P, 512], BF16)
        nc.vector.memset(warmsrc, 0.0)
        # force the exp activation table load at t=0, off the critical path
        tscr = const.tile([P, 1], F32)
        nc.vector.memset(tscr, 0.0)
        nc.scalar.activation(out=tscr, in_=tscr, func=ACT_EXP, bias=0.0, scale=1.0)
        # force the partition_broadcast Q7 library load at t=0 as well --
        # lazily it costs ~10.5us right at the stage-1 -> stage-2 handoff.
        brow = const.tile([1, 128], F32)
        nc.vector.memset(brow, 0.0)
        bwarm = const.tile([P, 128], F32)
        nc.gpsimd.partition_broadcast(bwarm, brow)

        # activations that persist across stages
        persist = ctx.enter_context(tc.tile_pool(name="persist", bufs=1))
        qt_sb = persist.tile([P, HPC, n_seq], BF16)    # Q^T  [d, h, n]
        kt_sb = persist.tile([P, HPC, n_seq], BF16)    # K^T  [d, h, n]
        v_sb = persist.tile([P, NT, DC], BF16)         # V natural [n(128), nt, d]
        ctxT_sb = persist.tile([P, HPC, n_seq], BF16)  # ctx^T [d, h, n]

        # scores psum pool spans both stages (4 banks)
        s_pool = ctx.enter_context(tc.tile_pool(name="sps", bufs=2, space="PSUM"))
        # ptg pool for groups 0/1 spans both stages (6 x 8KB)
        att1 = ctx.enter_context(tc.tile_pool(name="att1", bufs=6))

        ptgs = {}

        def emit_scores(g, h, pool):
            """S^T = K^T_kt.T @ Q^T per k tile, mask diagonal, exp into the
            P^T group tile.  Off-diagonal blocks run in pairs through a
            [128, 2, 512] two-bank psum tile and one ACTIVATE; the 4
            diagonal blocks share pair tiles half/half."""
            nkt = 4 * (g + 1)
            q0 = 4 * g * P
            ptg = pool.tile([P, nkt, 512], BF16, tag="ptg",
                            name=f"ptg_{g}_{h}")
            for kt0 in range(0, nkt, 2):
                sps = s_pool.tile([P, 2, 512], F32, tag="s", name="sps")
                if kt0 + 1 < 4 * g:
                    for j in (0, 1):
                        nc.tensor.matmul(
                            sps[:, j, :],
                            lhsT=kt_sb[:, h, (kt0 + j) * P:(kt0 + j + 1) * P],
                            rhs=qt_sb[:, h, q0:q0 + 512],
                            start=True, stop=True,
                        )
                    nc.scalar.activation(
                        out=ptg[:, kt0:kt0 + 2, :], in_=sps,
                        func=ACT_EXP, bias=0.0, scale=SCALE,
                    )
                else:
                    for j in (0, 1):
                        kt = kt0 + j
                        off = max(kt - 4 * g, 0) * P
                        width = 512 - off
                        nc.tensor.matmul(
                            sps[:, j, :width],
                            lhsT=kt_sb[:, h, kt * P:(kt + 1) * P],
                            rhs=qt_sb[:, h, q0 + off:q0 + 512],
                            start=True, stop=True,
                        )
                        if kt >= 4 * g:
                            nc.vector.tensor_tensor(
                                out=sps[:, j, :P], in0=sps[:, j, :P],
                                in1=mask2, op=ALU.add,
                            )
                        nc.scalar.activation(
                            out=ptg[:, kt, off:512], in_=sps[:, j, :width],
                            func=ACT_EXP, bias=0.0, scale=SCALE,
                        )
            return ptg

        def emit_sp(g, hp, pool):
            ptgs[(g, hp)] = emit_scores(g, hp, pool)
            ptgs[(g, hp + 1)] = emit_scores(g, hp + 1, pool)

        # ---------------- stage 1: qkv projection ----------------
        with tc.tile_pool(name="xw", bufs=1) as xw_pool, \
             tc.tile_pool(name="qkps", bufs=2, space="PSUM") as qk_pool, \
             tc.tile_pool(name="vps", bufs=2, space="PSUM") as v_pool:
            wq_sb = xw_pool.tile([P, NI, DC], BF16)
            wk_sb = xw_pool.tile([P, NI, DC], BF16)
            wv_sb = xw_pool.tile([P, NI, DC], BF16)
            # Dummy matmuls with no input dependencies: ramp the HAM clock
            # gate while the first DMAs land.
            warm_ps = v_pool.tile([P, 512], F32, tag="v", name="warm_ps")
            for _ in range(16):
                nc.tensor.matmul(warm_ps, lhsT=warmsrc[:, :P], rhs=warmsrc,
                                 start=True, stop=True)

            # Everything is partition-major in HBM, so each transfer below is
            # one dma_start whose per-partition runs are fully contiguous.
            xt_r = xt_d.rearrange("(q p) (io c) -> q p io c", q=NQ, io=NI)
            wq_r = wq_d.rearrange("p (io c) -> p io c", io=NI)
            wk_r = wk_d.rearrange("p (io c) -> p io c", io=NI)
            wv_r = wv_d.rearrange("p (io c) -> p io c", io=NI)

            xtq = [None] * NQ

            def xq_dma(q):
                t = xw_pool.tile([P, NI, 512], BF16, tag="xtq", bufs=2,
                                 name=f"xtq{q}")
                nc.sync.dma_start(t, xt_r[q])
                xtq[q] = t

            # DMA in consumption order (HWDGE ring drains FIFO).  wq and the
            # first x quarter are interleaved in 4-chunk sub-transfers so the
            # first QK psum group progresses chunk-by-chunk as data lands.
            t0 = xw_pool.tile([P, NI, 512], BF16, tag="xtq", bufs=2,
                              name="xtq0")
            xtq[0] = t0
            for i4 in range(0, NI, 4):
                s4 = slice(i4, i4 + 4)
                nc.sync.dma_start(wq_sb[:, s4, :], wq_r[:, s4, :])
                nc.sync.dma_start(t0[:, s4, :], xt_r[0, :, s4, :])
            nc.sync.dma_start(wk_sb, wk_r)
            nc.sync.dma_start(wv_sb, wv_r)
            for q in range(1, NQ):
                xq_dma(q)

            for q4 in range(NQ):
                xq = xtq[q4]
                sl = slice(q4 * 512, (q4 + 1) * 512)
                # Q^T / K^T for this quarter
                for w_sb, dst in ((wq_sb, qt_sb), (wk_sb, kt_sb)):
                    for h in range(HPC):
                        ps = qk_pool.tile([P, 512], F32, tag="qk")
                        for i in range(NI):
                            nc.tensor.matmul(
                                ps,
                                lhsT=w_sb[:, i, h * P:(h + 1) * P],
                                rhs=xq[:, i, :],
                                start=(i == 0), stop=(i == NI - 1),
                            )
                        nc.vector.tensor_copy(out=dst[:, h, sl], in_=ps)
                # V seq tiles of this quarter
                for j in range(4):
                    nt = 4 * q4 + j
                    ps = v_pool.tile([P, DC], F32, tag="v")
                    for i in range(NI):
                        nc.tensor.matmul(
                            ps,
                            lhsT=xq[:, i, j * P:(j + 1) * P],
                            rhs=wv_sb[:, i, :],
                            start=(i == 0), stop=(i == NI - 1),
                        )
                    nc.vector.tensor_copy(out=v_sb[:, nt, :], in_=ps)
                # early scores for groups 0/1: their exp work runs on
                # ScalarE underneath the remaining QK/V matmuls.
                if q4 == 0:
                    emit_sp(0, 0, att1)
                elif q4 == 1:
                    emit_sp(0, 2, att1)
                elif q4 == 2:
                    emit_sp(1, 0, att1)

        # ---------------- stage 2: attention drains + out-projection ------
        with tc.tile_pool(name="att2", bufs=4) as att2, \
             tc.tile_pool(name="att_small", bufs=2) as small_pool, \
             tc.tile_pool(name="colps", bufs=1, space="PSUM") as col_pool, \
             tc.tile_pool(name="cps", bufs=1, space="PSUM") as c_pool, \
             tc.tile_pool(name="out_sb", bufs=3) as out_pool, \
             tc.tile_pool(name="wo_sb", bufs=1) as wo_pool, \
             tc.tile_pool(name="ops", bufs=2, space="PSUM") as o_pool:
            wo_sb = wo_pool.tile([P, HPC, D_IN], BF16)
            wo_r = wo_d.rearrange("p (h j) -> p h j", h=HPC)
            nc.sync.dma_start(wo_sb, wo_r)
            out_r = out_d.rearrange("(nt jc p) c -> nt jc p c", nt=NT, jc=NJ)

            def emit_dr(g, hp):
                """Drain a head pair: sequential ones-row colsum chains in
                one PSUM bank, reciprocal, partition-broadcast, then ctx
                accumulation with the 1/sum fused into the PSUM->SBUF copy."""
                nkt = 4 * (g + 1)
                colp = col_pool.tile([P, 512], F32, tag="col", name="colp")
                btile = o_pool.tile([P, 512], F32, tag="o", name="colp_b")
                pa, pb = ptgs.pop((g, hp)), ptgs.pop((g, hp + 1))
                # col-tiled chains in separate banks: A -> col group 0 /
                # partition 0, B -> col group 1 / partition 32; interleaved
                # kt-major so the two chains stream concurrently.
                # (reciprocal_approx_fast garbles partition-offset inputs, so
                # chain B's row is tensor_copy'd to partition 0 first.)
                for kt in range(nkt):
                    off = max(kt - 4 * g, 0) * P
                    nc.tensor.matmul(
                        colp[0:1, off:512],
                        lhsT=ones_sb, rhs=pa[:, kt, off:512],
                        start=(kt == 0), stop=(kt == nkt - 1),
                        skip_group_check=True, tile_position=(0, 0),
                    )
                    nc.tensor.matmul(
                        btile[32:33, off:512],
                        lhsT=ones_sb, rhs=pb[:, kt, off:512],
                        start=(kt == 0), stop=(kt == nkt - 1),
                        skip_group_check=True, tile_position=(0, 32),
                    )
                bcs = []
                for cg, dst in ((0, colp), (1, btile)):
                    recip = small_pool.tile([1, 512], F32, tag="rsb",
                                            name="recip_sb")
                    if cg == 0:
                        nc.vector.reciprocal_approx_fast(out=recip,
                                                         in_=dst[0:1, :])
                    else:
                        nc.vector.tensor_copy(out=recip, in_=dst[32:33, :])
                        nc.vector.reciprocal_approx_fast(out=recip, in_=recip)
                    bc = small_pool.tile([P, 512], F32, tag="rbc",
                                         name="recip_bc")
                    nc.gpsimd.partition_broadcast(bc, recip)
                    bcs.append(bc)
                for h, pt, bc in ((hp, pa, bcs[0]), (hp + 1, pb, bcs[1])):
                    cps = c_pool.tile([P, 512], F32, tag="c", name="cps")
                    for kt in range(nkt):
                        off = max(kt - 4 * g, 0) * P
                        nc.tensor.matmul(
                            cps[:, off:512],
                            lhsT=v_sb[:, kt, h * P:(h + 1) * P],
                            rhs=pt[:, kt, off:512],
                            start=(kt == 0), stop=(kt == nkt - 1),
                            skip_group_check=True,
                        )
                    nc.vector.tensor_tensor(
                        out=ctxT_sb[:, h, g * 512:(g + 1) * 512],
                        in0=cps, in1=bc, op=ALU.mult,
                    )

            def emit_op(nts):
                """Partial out-projection for seq tiles `nts`; PSUM
                evacuation alternates VectorE / ScalarE; bf16 out."""
                for idx, nt in enumerate(nts):
                    for jc in range(NJ):
                        ops = o_pool.tile([P, 512], F32, tag="o", name="ops")
                        for hh in range(HPC):
                            nc.tensor.matmul(
                                ops,
                                lhsT=ctxT_sb[:, hh, nt * P:(nt + 1) * P],
                                rhs=wo_sb[:, hh, jc * 512:(jc + 1) * 512],
                                start=(hh == 0), stop=(hh == HPC - 1),
                            )
                        osb = out_pool.tile([P, 512], BF16, tag="osb",
                                            name="osb")
                        if (idx * NJ + jc) % 2:
                            nc.scalar.copy(out=osb, in_=ops)
                        else:
                            nc.vector.tensor_copy(out=osb, in_=ops)
                        nc.sync.dma_start(out_r[nt, jc], osb)

            # Explicit stage-2 schedule: drains for groups 0/1 run
            # immediately (their exps are already done); score pairs for
            # groups 2/3 and out-proj chunks are interleaved so every
            # drain's exp work is covered by preceding PE work.
            emit_dr(0, 0)
            emit_dr(0, 2)
            emit_sp(1, 2, att1)
            emit_op(range(0, 4))
            emit_sp(2, 0, att2)
            emit_dr(1, 0)
            emit_sp(2, 2, att2)
            emit_dr(1, 2)
            emit_op(range(4, 6))
            emit_sp(3, 0, att2)
            emit_dr(2, 0)
            emit_sp(3, 2, att2)
            emit_dr(2, 2)
            emit_op(range(8, 12))
            emit_dr(3, 0)
            emit_op(range(6, 8))
            emit_dr(3, 2)
            emit_op(range(12, 16))


def build_module(n_seq=N_SEQ):
    """Build and compile the per-core Bass module (SPMD: same program, 8 cores)."""
    nc = bacc.Bacc("TRN2", target_bir_lowering=False, debug=False,
                   num_devices=N_CORES)
    NI = D_IN // P
    xt_d = nc.dram_tensor("xt", [NQ * P, NI * 512], BF16,
                          kind="ExternalInput").ap()
    wq_d = nc.dram_tensor("wq", [P, NI * DC], BF16, kind="ExternalInput").ap()
    wk_d = nc.dram_tensor("wk", [P, NI * DC], BF16, kind="ExternalInput").ap()
    wv_d = nc.dram_tensor("wv", [P, NI * DC], BF16, kind="ExternalInput").ap()
    wo_d = nc.dram_tensor("wo", [P, HPC * D_IN], BF16, kind="ExternalInput").ap()
    out_d = nc.dram_tensor("out", [n_seq * (D_IN // 512), 512], BF16,
                           kind="ExternalOutput").ap()
    with tile.TileContext(nc) as tc:
        _build_body(tc, xt_d, wq_d, wk_d, wv_d, wo_d, out_d, n_seq)
    nc.compile()
    return nc


def make_in_maps(x, W_qkv, W_out):
    """Host-side sharding: per-core input dict, bf16 cast.  x is blocked as
    [quarter q][chunk io][128 p][512 c] = x[b].T[io*128+p, q*512+c] so every
    stage-1 DMA descriptor is one contiguous 128KB block."""
    bf = ml_dtypes.bfloat16
    NI = D_IN // P

    def wblk(w):
        # [d_in, c] -> [p, io*c]: partition-major, contiguous per partition
        return np.ascontiguousarray(
            w.reshape(NI, P, -1).transpose(1, 0, 2).reshape(P, -1)).astype(bf)

    in_maps = []
    for c in range(N_CORES):
        b, g = divmod(c, 4)
        cs = slice(DC * g, DC * (g + 1))
        xT = np.asarray(x[b]).T.astype(bf)                    # [d, n]
        # [q*p, io*c]: quarter-then-partition major
        xq = xT.reshape(NI, P, NQ, 512).transpose(2, 1, 0, 3).reshape(NQ * P, -1)
        wo = np.asarray(W_out[cs, :]).reshape(HPC, P, D_IN).transpose(1, 0, 2)
        in_maps.append({
            "xt": np.ascontiguousarray(xq),
            "wq": wblk(W_qkv[:, 0 * D_IN:1 * D_IN][:, cs]),
            "wk": wblk(W_qkv[:, 1 * D_IN:2 * D_IN][:, cs]),
            "wv": wblk(W_qkv[:, 2 * D_IN:3 * D_IN][:, cs]),
            "wo": np.ascontiguousarray(wo.reshape(P, -1)).astype(bf),
        })
    return in_maps


_NC_CACHE = {}


def get_module():
    if "nc" not in _NC_CACHE:
        _NC_CACHE["nc"] = build_module()
    return _NC_CACHE["nc"]


def run(x, W_qkv, W_out, b_out, trace=False, **trace_kwargs):
    nc = get_module()
    in_maps = make_in_maps(x, W_qkv, W_out)
    res = run_bass_kernel_spmd(nc, in_maps, core_ids=list(range(N_CORES)),
                               trace=trace, **trace_kwargs)
    NT, NJ = N_SEQ // P, D_IN // 512
    parts = np.stack([
        np.asarray(res.results[c]["out"], dtype=np.float32)
        .reshape(NT, NJ, P, 512).transpose(0, 2, 1, 3).reshape(N_SEQ, D_IN)
        for c in range(N_CORES)])
    parts = parts.reshape(2, 4, N_SEQ, D_IN)
    out = parts.sum(axis=1, dtype=np.float64).astype(np.float32)
    out += b_out.astype(np.float32)
    return out, res


def kernel(x, W_qkv, W_out, b_out):
    out, _ = run(np.asarray(x), np.asarray(W_qkv), np.asarray(W_out),
                 np.asarray(b_out))
    return out
